# revision 37
# baseline (speedup 1.0000x reference)
"""Trainium2 Bass kernel for nn_CrossAttention_59717225284223.

Full-input contract: kernel(**inputs) takes the complete [4,256,8192] tensors,
shards across 8 NeuronCores internally (core i -> batch i//2, N-half i%2; the
x2/KV side is recomputed per batch pair so no collectives are needed), and
returns the full [4,256,8192] float32 output.

v2: bf16 matmul/elementwise datapath, bias folds (K/V bias via rank-1
extraction fold, q bias via activation bias), LN2 mean-centering and g2
folded into W2 on the host, x1 residual folded into the aug conv, and
elementwise work balanced across Vector/Scalar/GpSimd engines.
"""

import os
import sys

import numpy as np

for _p in (
    "/root/.axon_site",
    "/root/.axon_site/_ro/trn_rl_repo",
    "/opt/trn_rl_repo",
):
    if os.path.isdir(_p) and _p not in sys.path:
        sys.path.append(_p)

import ml_dtypes  # noqa: E402

import concourse.bass as bass  # noqa: E402
import concourse.tile as tile  # noqa: E402
from concourse import bacc, mybir  # noqa: E402
from concourse.bass_utils import run_bass_kernel_spmd  # noqa: E402

F32 = mybir.dt.float32
F32R = mybir.dt.float32r
BF16 = mybir.dt.bfloat16
FP8 = mybir.dt.float8e4
OP = mybir.AluOpType
AF = mybir.ActivationFunctionType
PM_DR = mybir.MatmulPerfMode.DoubleRow
BFNP = ml_dtypes.bfloat16
F8NP = ml_dtypes.float8_e4m3
W1SCALE = 16.0

B, C, N = 4, 256, 8192
H, D = 4, 64
L = N // 2          # positions per core
NT = N // 128       # x2-side 128-position tiles
NCH = L // 512      # 512-position chunks per core
LN_EPS = 1e-5
BN_EPS = 1e-5
ATTN_EPS = 1e-6

_CACHE = {}


def _build():
    nc = bacc.Bacc(None, target_bir_lowering=False)

    x1 = nc.dram_tensor("x1", [C, L], BF16, kind="ExternalInput")
    x18 = nc.dram_tensor("x18", [C, L], FP8, kind="ExternalInput")
    x2 = nc.dram_tensor("x2", [C, N], FP8, kind="ExternalInput")
    wkv = nc.dram_tensor("wkv", [C, 2 * C], FP8, kind="ExternalInput")
    wq = nc.dram_tensor("wq", [C, C], FP8, kind="ExternalInput")
    wfin = nc.dram_tensor("wfin", [C, C], BF16, kind="ExternalInput")
    w1a = nc.dram_tensor("w1a", [C, 2 * C], BF16, kind="ExternalInput")
    w1b = nc.dram_tensor("w1b", [C, 2 * C], BF16, kind="ExternalInput")
    w2 = nc.dram_tensor("w2", [2 * C, C], BF16, kind="ExternalInput")
    biasks = nc.dram_tensor("biasks", [1, C], BF16, kind="ExternalInput")
    bq1row = nc.dram_tensor("bq1row", [1, C], BF16, kind="ExternalInput")
    onesr512 = nc.dram_tensor("onesr512", [1, 512], BF16, kind="ExternalInput")
    bvrow = nc.dram_tensor("bvrow", [1, C], BF16, kind="ExternalInput")
    bq = nc.dram_tensor("bq", [C, 1], F32, kind="ExternalInput")
    bq1 = nc.dram_tensor("bq1", [C, 1], F32, kind="ExternalInput")
    hbv = nc.dram_tensor("hb", [2 * C, 1], F32, kind="ExternalInput")
    ba = nc.dram_tensor("ba", [C, 1], F32, kind="ExternalInput")
    invc = nc.dram_tensor("invc", [C, 1], BF16, kind="ExternalInput")
    thr = nc.dram_tensor("thr", [1, 1], F32, kind="ExternalInput")
    ident = nc.dram_tensor("ident", [128, 128], BF16, kind="ExternalInput")
    ones2d = nc.dram_tensor("ones2d", [128, 2], BF16, kind="ExternalInput")
    onesrow = nc.dram_tensor("onesrow", [1, 128], F32, kind="ExternalInput")
    zbd = nc.dram_tensor("zbd", [C, 260], BF16, kind="ExternalInput")
    out = nc.dram_tensor("out", [C, L], F32, kind="ExternalOutput")

    x1r = x1[:, :].rearrange("(t p) n -> p t n", p=128)
    x18r = x18[:, :].rearrange("(t p) n -> p t n", p=128)
    x2r = x2[:, :].rearrange("(t p) n -> p t n", p=128)
    outr = out[:, :].rearrange("(t p) n -> p t n", p=128)

    def bcast(ap, p):
        # partition-broadcast of a [1, F] DRAM row to [p, F]
        return bass.AP(tensor=ap.tensor, offset=ap.offset,
                       ap=[[0, p]] + [list(d) for d in ap.ap[1:]])

    with tile.TileContext(nc) as tc:
        with tc.tile_pool(name="consts", bufs=1) as consts, \
             tc.tile_pool(name="resident", bufs=1) as res:
            # ---- constants ----
            wkv_sb = consts.tile([128, 2, 2 * C], FP8)
            for t in range(2):
                nc.sync.dma_start(out=wkv_sb[:, t, :], in_=wkv[:, :].rearrange(
                    "(t p) o -> p t o", p=128)[:, t, :])
            wq_sb = consts.tile([128, 2, C], FP8)
            nc.sync.dma_start(out=wq_sb, in_=wq[:, :].rearrange(
                "(t p) o -> p t o", p=128))
            wfin_sb = consts.tile([128, 2, C], BF16)
            nc.sync.dma_start(out=wfin_sb, in_=wfin[:, :].rearrange(
                "(t p) o -> p t o", p=128))
            w1a_sb = consts.tile([128, 2, 2 * C], BF16)
            w1b_sb = consts.tile([128, 2, 2 * C], BF16)
            w2_sb = consts.tile([128, 4, C], BF16)
            for t in range(2):
                nc.sync.dma_start(out=w1a_sb[:, t, :], in_=w1a[:, :].rearrange(
                    "(t p) o -> p t o", p=128)[:, t, :])
                nc.sync.dma_start(out=w1b_sb[:, t, :], in_=w1b[:, :].rearrange(
                    "(t p) o -> p t o", p=128)[:, t, :])
            for t in range(4):
                nc.sync.dma_start(out=w2_sb[:, t, :], in_=w2[:, :].rearrange(
                    "(t p) o -> p t o", p=128)[:, t, :])
            ident_sb = consts.tile([128, 128], BF16)
            nc.sync.dma_start(out=ident_sb, in_=ident[:, :])

            bq_sb = consts.tile([128, 2], F32)
            bq1_sb = consts.tile([128, 2], F32)
            ba_sb = consts.tile([128, 2], F32)
            for t in range(2):
                sl = slice(t * 128, (t + 1) * 128)
                nc.sync.dma_start(out=bq_sb[:, t:t + 1], in_=bq[sl, :])
                nc.sync.dma_start(out=bq1_sb[:, t:t + 1], in_=bq1[sl, :])
                nc.sync.dma_start(out=ba_sb[:, t:t + 1], in_=ba[sl, :])
            hb_sb = consts.tile([128, 4], F32)
            for t in range(4):
                nc.sync.dma_start(out=hb_sb[:, t:t + 1],
                                  in_=hbv[t * 128:(t + 1) * 128, :])
            invc_sb = consts.tile([128, 2], BF16)
            for t in range(2):
                nc.sync.dma_start(out=invc_sb[:, t:t + 1],
                                  in_=invc[t * 128:(t + 1) * 128, :])
            biasks_sb = consts.tile([1, C], BF16)
            nc.sync.dma_start(out=biasks_sb, in_=biasks[:, :])
            bq1row_sb = consts.tile([1, C], BF16)
            nc.sync.dma_start(out=bq1row_sb, in_=bq1row[:, :])
            onesr512_sb = consts.tile([1, 512], BF16)
            nc.sync.dma_start(out=onesr512_sb, in_=onesr512[:, :])
            bvbc = consts.tile([128, C], BF16)
            nc.gpsimd.dma_start(out=bvbc, in_=bcast(bvrow[:, :], 128))
            thr_bc = consts.tile([128, 1], F32)
            nc.gpsimd.dma_start(out=thr_bc, in_=bcast(thr[:, :], 128))
            onesrow_sb = consts.tile([1, 128], F32R)
            nc.sync.dma_start(out=onesrow_sb,
                              in_=onesrow[:, :].bitcast(F32R))
            ones2_sb = consts.tile([128, 2], BF16)
            nc.sync.dma_start(out=ones2_sb, in_=ones2d[:, :])
            lneps = consts.tile([128, 1], F32)
            nc.vector.memset(lneps, LN_EPS)
            eps11 = consts.tile([1, 1], F32)
            nc.vector.memset(eps11, LN_EPS)
            neg1 = consts.tile([128, 1], F32)
            nc.vector.memset(neg1, -1.0)
            l16m1 = consts.tile([128, 1], F32)
            nc.vector.memset(l16m1, 1.7725887222397811)

            # ---- resident activations ----
            x1_sb = res.tile([128, 2, L], BF16)
            x18_sb = res.tile([128, 2, L], FP8)
            q_sb = res.tile([128, 2, L], BF16)
            msgn_sb = res.tile([128, 2, L], BF16)
            kvbd = res.tile([128, 2, 260], BF16)

            # ================= phase 1: x2 side (full N) =================
            with tc.tile_pool(name="x2p", bufs=3) as x2p, \
                 tc.tile_pool(name="kvbp", bufs=1) as kvbp, \
                 tc.tile_pool(name="sc1", bufs=6) as sc1, \
                 tc.tile_pool(name="cps", bufs=4, space="PSUM") as cps, \
                 tc.tile_pool(name="kvps", bufs=1, space="PSUM") as kvps:
                kv_ps = [kvps.tile([128, 258], F32, name=f"kv_ps{m}", tag=f"kv{m}")
                         for m in range(2)]
                for ch in range(N // 512):
                    x2t = x2p.tile([128, 2, 512], FP8)
                    for t in range(2):
                        nc.sync.dma_start(
                            out=x2t[:, t, :],
                            in_=x2r[:, t, ch * 512:(ch + 1) * 512])
                    for s in range(4):
                        i = ch * 4 + s
                        # cp = 16*(conv(x2) + [bk+1 | 0])  (fp8 DoubleRow)
                        cp = cps.tile([128, 2 * C], F32)
                        nc.tensor.matmul(cp, x2t[:, :, s * 128:(s + 1) * 128],
                                         wkv_sb[:, :, :], start=True,
                                         stop=True, perf_mode=PM_DR)
                        nc.tensor.matmul(cp[:, 0:256], onesr512_sb[:, 0:128],
                                         biasks_sb, start=False, stop=True,
                                         skip_group_check=True)
                        kvb = kvbp.tile([128, 514], BF16, name=f"kvbu{i}",
                                        tag=f"kvb{i % 8}")
                        nc.gpsimd.tensor_copy(out=kvb[:, 512:514],
                                              in_=ones2_sb)
                        # K16 = max(cp, min(16*exp(cp/16 - 1), 16))
                        ex = sc1.tile([128, C], BF16)
                        nc.scalar.activation(out=ex, in_=cp[:, 0:256],
                                             func=AF.Exp, bias=l16m1,
                                             scale=0.0625)
                        nc.vector.scalar_tensor_tensor(
                            out=kvb[:, 0:256], in0=ex, scalar=16.0,
                            in1=cp[:, 0:256], op0=OP.min, op1=OP.max)
                        # V half: scale back by 1/16
                        if i % 2 == 0:
                            nc.scalar.activation(out=kvb[:, 256:512],
                                                 in_=cp[:, 256:512],
                                                 func=AF.Copy, scale=0.0625)
                        else:
                            nc.vector.tensor_scalar(
                                out=kvb[:, 256:512], in0=cp[:, 256:512],
                                scalar1=0.0625, scalar2=None, op0=OP.mult)
                        nc.tensor.matmul(kv_ps[0], kvb[:, 0:128],
                                         kvb[:, 256:514],
                                         start=(i == 0), stop=(i == NT - 1))
                        nc.tensor.matmul(kv_ps[1], kvb[:, 128:256],
                                         kvb[:, 256:514],
                                         start=(i == 0), stop=(i == NT - 1))

                # ---- KVbd / KsumBD extraction (V bias folded as Ksum x bv) ----
                nc.sync.dma_start(out=kvbd, in_=zbd[:, :].rearrange(
                    "(t p) o -> p t o", p=128))
                ksum_sb = sc1.tile([128, 2], F32)
                for t in range(2):
                    nc.vector.tensor_copy(out=ksum_sb[:, t:t + 1],
                                          in_=kv_ps[t][:, 256:257])
                for t in range(2):
                    for hh in range(2):
                        h = t * 2 + hh
                        rsl = slice(hh * 64, hh * 64 + 64)
                        csl = slice(h * 64, h * 64 + 64)
                        nc.vector.scalar_tensor_tensor(
                            out=kvbd[rsl, t, csl], in0=bvbc[rsl, csl],
                            scalar=ksum_sb[rsl, t:t + 1],
                            in1=kv_ps[t][rsl, csl],
                            op0=OP.mult, op1=OP.add)
                        nc.vector.tensor_copy(
                            out=kvbd[rsl, t, 256 + h:257 + h],
                            in_=kv_ps[t][rsl, 256:257])

            # ============ phase 2+3: q conv, msg, LN1 ============
            with tc.tile_pool(name="sc2", bufs=4) as sc2, \
                 tc.tile_pool(name="sc3", bufs=6) as sc3, \
                 tc.tile_pool(name="stat", bufs=4) as stat, \
                 tc.tile_pool(name="qaps", bufs=2, space="PSUM") as qaps, \
                 tc.tile_pool(name="msgps", bufs=1, space="PSUM") as msgps, \
                 tc.tile_pool(name="trps", bufs=2, space="PSUM") as trps:
                for ch in range(NCH):
                    sl = slice(ch * 512, (ch + 1) * 512)
                    for t in range(2):
                        nc.sync.dma_start(out=x1_sb[:, t, sl],
                                          in_=x1r[:, t, sl])
                        nc.sync.dma_start(out=x18_sb[:, t, sl],
                                          in_=x18r[:, t, sl])
                    for m in range(2):
                        mc = slice(m * 128, (m + 1) * 128)
                        # qp = 16*(conv(x1) + bq + 1)  (fp8 DoubleRow)
                        qp = qaps.tile([128, 512], F32)
                        nc.tensor.matmul(qp, wq_sb[:, :, mc],
                                         x18_sb[:, :, sl], start=True,
                                         stop=True, perf_mode=PM_DR)
                        nc.tensor.matmul(qp, bq1row_sb[:, mc], onesr512_sb,
                                         start=False, stop=True,
                                         skip_group_check=True)
                        ex2 = sc2.tile([128, 512], BF16)
                        nc.scalar.activation(out=ex2, in_=qp, func=AF.Exp,
                                             bias=l16m1, scale=0.0625)
                        nc.vector.scalar_tensor_tensor(
                            out=q_sb[:, m, sl], in0=ex2, scalar=16.0, in1=qp,
                            op0=OP.min, op1=OP.max)
                    zsc = stat.tile([128, 16], F32, name="zsc", tag="zsc")
                    mps = []
                    for s_ in range(4):
                        l0 = ch * 512 + s_ * 128
                        lsl = slice(l0, l0 + 128)
                        mp = msgps.tile([128, 260], F32, name=f"mp{s_}",
                                        tag=f"mp{s_}")
                        nc.tensor.matmul(mp, q_sb[:, 0, lsl], kvbd[:, 0, :],
                                         start=True, stop=False)
                        nc.tensor.matmul(mp, q_sb[:, 1, lsl], kvbd[:, 1, :],
                                         start=False, stop=True)
                        nc.vector.tensor_copy(out=zsc[:, s_ * 4:s_ * 4 + 4],
                                              in_=mp[:, 256:260])
                        mps.append(mp)
                    mk = stat.tile([128, 16], F32, name="mk", tag="mk")
                    nc.vector.tensor_scalar(
                        out=mk, in0=zsc, scalar1=thr_bc,
                        scalar2=None, op0=OP.is_gt)
                    nc.vector.tensor_tensor(out=mk, in0=zsc, in1=mk,
                                            op=OP.mult)
                    nc.vector.tensor_scalar(
                        out=mk, in0=mk, scalar1=ATTN_EPS, scalar2=None,
                        op0=OP.add)
                    zt = stat.tile([128, 16], F32, name="zt", tag="zt")
                    nc.vector.reciprocal(out=zt, in_=mk)
                    sums = stat.tile([128, 4], F32, name="sums", tag="sums")
                    sumsq = stat.tile([128, 4], F32, name="sumsq", tag="sumsq")
                    mss = []
                    for s_ in range(4):
                        mp = mps[s_]
                        ms = sc3.tile([128, C], BF16, name="ms", tag=f"ms{s_}")
                        zb = zt[:, s_ * 4:s_ * 4 + 4]
                        zb = bass.AP(tensor=zb.tensor, offset=zb.offset,
                                     ap=[list(zb.ap[0]), list(zb.ap[1]),
                                         [0, 64]])
                        nc.vector.scalar_tensor_tensor(
                            out=ms.rearrange("p (h d) -> p h d", h=4),
                            in0=mp[:, 0:256].rearrange("p (h d) -> p h d", h=4),
                            scalar=0.0, in1=zb, op0=OP.add, op1=OP.mult,
                            accum_out=sums[:, s_:s_ + 1])
                        sqj = sc3.tile([128, C], BF16, name="sqj", tag="sqj")
                        nc.scalar.activation(out=sqj, in_=ms, func=AF.Square,
                                             accum_out=sumsq[:, s_:s_ + 1])
                        mss.append(ms)
                    mu = stat.tile([128, 4], F32, name="mu", tag="mu")
                    nc.vector.tensor_scalar(out=mu, in0=sums, scalar1=1.0 / C,
                                            scalar2=None, op0=OP.mult)
                    musq = stat.tile([128, 4], F32, name="musq", tag="musq")
                    nc.vector.tensor_tensor(out=musq, in0=mu, in1=mu,
                                            op=OP.mult)
                    varq = stat.tile([128, 4], F32, name="varq", tag="varq")
                    nc.vector.scalar_tensor_tensor(
                        out=varq, in0=sumsq, scalar=1.0 / C, in1=musq,
                        op0=OP.mult, op1=OP.subtract)
                    sdv = stat.tile([128, 4], F32, name="sdv", tag="sdv")
                    nc.scalar.activation(out=sdv, in_=varq, func=AF.Sqrt,
                                         bias=lneps)
                    rz = stat.tile([128, 4], F32, name="rz", tag="rz")
                    nc.vector.reciprocal(out=rz, in_=sdv)
                    for s_ in range(4):
                        l0 = ch * 512 + s_ * 128
                        lsl = slice(l0, l0 + 128)
                        msn = sc3.tile([128, C], BF16, name="msn", tag="msn")
                        nc.vector.tensor_scalar(
                            out=msn, in0=mss[s_], scalar1=mu[:, s_:s_ + 1],
                            scalar2=rz[:, s_:s_ + 1],
                            op0=OP.subtract, op1=OP.mult)
                        for t in range(2):
                            tp = trps.tile([128, 128], BF16)
                            nc.tensor.transpose(
                                tp, msn[:, t * 128:(t + 1) * 128], ident_sb)
                            if t == 0:
                                nc.scalar.copy(out=msgn_sb[:, t, lsl], in_=tp)
                            else:
                                nc.vector.tensor_copy(out=msgn_sb[:, t, lsl],
                                                      in_=tp)

            # ================= phase 4: MLP + LN2 + final =================
            with tc.tile_pool(name="hpool", bufs=5) as hpool, \
                 tc.tile_pool(name="sc4", bufs=4) as sc4, \
                 tc.tile_pool(name="st4", bufs=2) as st4, \
                 tc.tile_pool(name="outp", bufs=3) as outp, \
                 tc.tile_pool(name="hps", bufs=2, space="PSUM") as hps, \
                 tc.tile_pool(name="o2ps", bufs=1, space="PSUM") as o2ps, \
                 tc.tile_pool(name="stps", bufs=1, space="PSUM") as stps, \
                 tc.tile_pool(name="rbps", bufs=1, space="PSUM") as rbps, \
                 tc.tile_pool(name="augps", bufs=2, space="PSUM") as augps:
                for ch in range(NCH):
                    sl = slice(ch * 512, (ch + 1) * 512)
                    hsb = []
                    for m in range(4):
                        mc = slice(m * 128, (m + 1) * 128)
                        hp = hps.tile([128, 512], F32)
                        nc.tensor.matmul(hp, w1a_sb[:, 0, mc], x1_sb[:, 0, sl],
                                         start=True, stop=False)
                        nc.tensor.matmul(hp, w1a_sb[:, 1, mc], x1_sb[:, 1, sl],
                                         start=False, stop=False)
                        nc.tensor.matmul(hp, w1b_sb[:, 0, mc],
                                         msgn_sb[:, 0, sl],
                                         start=False, stop=False)
                        nc.tensor.matmul(hp, w1b_sb[:, 1, mc],
                                         msgn_sb[:, 1, sl],
                                         start=False, stop=True)
                        ht = hpool.tile([128, 512], BF16)
                        nc.scalar.activation(out=ht, in_=hp, func=AF.Relu,
                                             bias=hb_sb[:, m:m + 1])
                        hsb.append(ht)
                    o2p = [o2ps.tile([128, 512], F32, name=f"o2p{m2}",
                                     tag=f"o2_{m2}")
                           for m2 in range(2)]
                    for m2 in range(2):
                        mc2 = slice(m2 * 128, (m2 + 1) * 128)
                        for k in range(4):
                            nc.tensor.matmul(o2p[m2], w2_sb[:, k, mc2], hsb[k],
                                             start=(k == 0), stop=(k == 3))
                    # LN2: mean already folded into W2; var = sum(invc * z^2)
                    ssq = stps.tile([1, 512], F32, tag="ssq")
                    for m2 in range(2):
                        sqs = sc4.tile([128, 512], BF16, name=f"sqs{m2}",
                                       tag=f"sqs{m2}")
                        nc.scalar.activation(out=sqs, in_=o2p[m2],
                                             func=AF.Square)
                        nc.tensor.matmul(ssq, invc_sb[:, m2:m2 + 1], sqs,
                                         start=(m2 == 0), stop=(m2 == 1))
                    sd_row = st4.tile([1, 512], F32, name="sd_row", tag="sd")
                    nc.scalar.activation(out=sd_row, in_=ssq, func=AF.Sqrt,
                                         bias=eps11)
                    rstd_row = st4.tile([1, 512], F32, name="rstd_row",
                                        tag="rstd")
                    nc.vector.reciprocal_approx_fast(out=rstd_row, in_=sd_row)
                    rstd_r = st4.tile([1, 512], F32R, name="rstd_r",
                                      tag="rstd_r")
                    nc.vector.tensor_copy(out=rstd_r, in_=rstd_row)
                    rbc = rbps.tile([128, 512], F32, tag="rbc")
                    nc.tensor.matmul(rbc, onesrow_sb, rstd_r,
                                     start=True, stop=True)
                    rbc_sb = sc4.tile([128, 512], BF16, name="rbc_sb",
                                      tag="rbc_sb")
                    nc.vector.tensor_copy(out=rbc_sb, in_=rbc)
                    for m2 in range(2):
                        ap_ = augps.tile([128, 512], F32)
                        nc.tensor.matmul(ap_, wfin_sb[:, 0, m2 * 128:(m2 + 1) * 128],
                                         x1_sb[:, 0, sl], start=True, stop=False)
                        nc.tensor.matmul(ap_, wfin_sb[:, 1, m2 * 128:(m2 + 1) * 128],
                                         x1_sb[:, 1, sl], start=False, stop=True)
                        t1 = sc4.tile([128, 512], BF16, name=f"t1_{m2}",
                                      tag=f"t1_{m2}")
                        nc.vector.tensor_tensor(out=t1, in0=o2p[m2],
                                                in1=rbc_sb, op=OP.mult)
                        ot = outp.tile([128, 512], F32)
                        nc.vector.scalar_tensor_tensor(
                            out=ot, in0=ap_, scalar=ba_sb[:, m2:m2 + 1], in1=t1,
                            op0=OP.add, op1=OP.add)
                        nc.sync.dma_start(out=outr[:, m2, sl], in_=ot)

    nc.compile()
    return nc


def _host_prep(inputs):
    """Fold BN/LN affine params into weights; build per-core input maps."""
    f32 = np.float32
    x1 = np.asarray(inputs["x1"], f32)
    x2 = np.asarray(inputs["x2"], f32)
    Wq, bq = np.asarray(inputs["Wq"], f32), np.asarray(inputs["bq"], f32)
    Wk, bk = np.asarray(inputs["Wk"], f32), np.asarray(inputs["bk"], f32)
    Wv, bv = np.asarray(inputs["Wv"], f32), np.asarray(inputs["bv"], f32)
    W1, W2 = np.asarray(inputs["W1"], f32), np.asarray(inputs["W2"], f32)
    g1, b1 = np.asarray(inputs["g1"], f32), np.asarray(inputs["b1"], f32)
    g2, b2 = np.asarray(inputs["g2"], f32), np.asarray(inputs["b2"], f32)
    Wa, ba = np.asarray(inputs["Wa"], f32), np.asarray(inputs["ba"], f32)
    bn_g, bn_b = np.asarray(inputs["bn_g"], f32), np.asarray(inputs["bn_b"], f32)
    bn_m, bn_v = np.asarray(inputs["bn_m"], f32), np.asarray(inputs["bn_v"], f32)
    thr = np.asarray(inputs["threshold"], f32)

    c = lambda a: np.ascontiguousarray(a, dtype=f32)
    cb = lambda a: np.ascontiguousarray(np.asarray(a, f32), dtype=BFNP)
    c8 = lambda a: np.ascontiguousarray(np.asarray(a, f32), dtype=F8NP)

    wkv = np.concatenate([Wk.T, Wv.T], axis=1)               # [C, 2C]
    scale_bn = bn_g / np.sqrt(bn_v + BN_EPS)
    wfin = (scale_bn[:, None] * Wa).T + np.eye(C, dtype=f32)  # aug + x1 fold
    ba_f = scale_bn * ba + (bn_b - bn_m * scale_bn) + b2
    W1a, W1b = W1[:, :C], W1[:, C:]
    w1a = W1a.T                                              # [C, 2C]
    w1b = (W1b * g1[None, :]).T                              # [C, 2C]
    hb = (W1b @ b1)[:, None]                                 # [2C, 1]
    # W2 with LN2 mean-centering and g2 folded in (per output channel)
    w2t = W2.T                                               # [2C, C]
    w2pp = (w2t - w2t.mean(axis=1, keepdims=True)) * g2[None, :]
    invc = (1.0 / (C * g2 * g2))[:, None]                    # [C, 1]
    shared = {
        "wkv": c8(wkv * W1SCALE),
        "wq": c8(Wq.T * W1SCALE), "bq": c(bq[:, None]),
        "bq1": c(bq[:, None] + 1.0),
        "wfin": cb(wfin), "ba": c(ba_f[:, None]),
        "w1a": cb(w1a), "w1b": cb(w1b), "hb": c(hb),
        "w2": cb(w2pp),
        "biasks": cb(W1SCALE * (bk + 1.0)[None, :]), "bvrow": cb(bv[None, :]),
        "bq1row": cb(W1SCALE * (bq + 1.0)[None, :]),
        "onesr512": cb(np.ones((1, 512), dtype=f32)),
        "invc": cb(invc),
        "thr": c(thr.reshape(1, 1) * (W1SCALE * W1SCALE)),
        "ident": cb(np.eye(128, dtype=f32)),
        "ones2d": cb(np.ones((128, 2), dtype=f32)),
        "onesrow": c(np.ones((1, 128), dtype=f32)),
        "zbd": cb(np.zeros((C, 260), dtype=f32)),
    }
    x1b = np.ascontiguousarray(x1, dtype=BFNP)
    x18a = np.ascontiguousarray(x1, dtype=F8NP)
    x2b = np.ascontiguousarray(x2, dtype=F8NP)
    in_maps = []
    for core in range(8):
        b_, half = core // 2, core % 2
        m = dict(shared)
        m["x1"] = np.ascontiguousarray(x1b[b_][:, half * L:(half + 1) * L])
        m["x18"] = np.ascontiguousarray(x18a[b_][:, half * L:(half + 1) * L])
        m["x2"] = np.ascontiguousarray(x2b[b_])
        in_maps.append(m)
    return in_maps


def _get_nc():
    if "nc" not in _CACHE:
        _CACHE["nc"] = _build()
    return _CACHE["nc"]


def kernel(**inputs) -> np.ndarray:
    nc = _get_nc()
    in_maps = _host_prep(inputs)
    res = run_bass_kernel_spmd(nc, in_maps, core_ids=list(range(8)),
                               trace=bool(int(os.environ.get("KBENCH_TRACE", "0"))))
    if os.environ.get("KBENCH_TIME_OUT"):
        with open(os.environ["KBENCH_TIME_OUT"], "w") as f:
            f.write(str(res.exec_time_ns))
    out = np.empty((B, C, N), np.float32)
    for core in range(8):
        b_, half = core // 2, core % 2
        out[b_][:, half * L:(half + 1) * L] = res.results[core]["out"]
    return out


# revision 39
# speedup vs baseline: 1.2064x; 1.2064x over previous
"""Trainium2 Bass kernel for nn_CrossAttention_59717225284223.

Full-input contract: kernel(**inputs) takes the complete [4,256,8192] tensors,
shards across 8 NeuronCores internally (core i -> batch i//2, N-half i%2; the
x2/KV side is recomputed per batch pair so no collectives are needed), and
returns the full [4,256,8192] float32 output.

v2: bf16 matmul/elementwise datapath, bias folds (K/V bias via rank-1
extraction fold, q bias via activation bias), LN2 mean-centering and g2
folded into W2 on the host, x1 residual folded into the aug conv, and
elementwise work balanced across Vector/Scalar/GpSimd engines.
"""

import os
import sys

import numpy as np

for _p in (
    "/root/.axon_site",
    "/root/.axon_site/_ro/trn_rl_repo",
    "/opt/trn_rl_repo",
):
    if os.path.isdir(_p) and _p not in sys.path:
        sys.path.append(_p)

import ml_dtypes  # noqa: E402

import concourse.bass as bass  # noqa: E402
import concourse.tile as tile  # noqa: E402
from concourse import bacc, mybir  # noqa: E402
from concourse.bass_utils import run_bass_kernel_spmd  # noqa: E402

F32 = mybir.dt.float32
F32R = mybir.dt.float32r
BF16 = mybir.dt.bfloat16
FP8 = mybir.dt.float8e4
OP = mybir.AluOpType
AF = mybir.ActivationFunctionType
PM_DR = mybir.MatmulPerfMode.DoubleRow
BFNP = ml_dtypes.bfloat16
F8NP = ml_dtypes.float8_e4m3
W1SCALE = 16.0

B, C, N = 4, 256, 8192
H, D = 4, 64
L = N // 2          # positions per core
NT = N // 128       # x2-side 128-position tiles
NCH = L // 512      # 512-position chunks per core
LN_EPS = 1e-5
BN_EPS = 1e-5
ATTN_EPS = 1e-6

_CACHE = {}


def _build():
    nc = bacc.Bacc(None, target_bir_lowering=False, num_devices=8)

    x1 = nc.dram_tensor("x1", [C, L], BF16, kind="ExternalInput")
    x2 = nc.dram_tensor("x2", [C, L], BF16, kind="ExternalInput")
    wkv = nc.dram_tensor("wkv", [C, 2 * C], BF16, kind="ExternalInput")
    wq = nc.dram_tensor("wq", [C, C], BF16, kind="ExternalInput")
    wfin = nc.dram_tensor("wfin", [C, C], BF16, kind="ExternalInput")
    w1a = nc.dram_tensor("w1a", [C, 2 * C], BF16, kind="ExternalInput")
    w1b = nc.dram_tensor("w1b", [C, 2 * C], BF16, kind="ExternalInput")
    w2 = nc.dram_tensor("w2", [2 * C, C], BF16, kind="ExternalInput")
    biask1 = nc.dram_tensor("biask1", [1, C], BF16, kind="ExternalInput")
    bvrow = nc.dram_tensor("bvrow", [1, C], BF16, kind="ExternalInput")
    bq = nc.dram_tensor("bq", [C, 1], F32, kind="ExternalInput")
    bq1 = nc.dram_tensor("bq1", [C, 1], F32, kind="ExternalInput")
    hbv = nc.dram_tensor("hb", [2 * C, 1], F32, kind="ExternalInput")
    ba = nc.dram_tensor("ba", [C, 1], F32, kind="ExternalInput")
    invc = nc.dram_tensor("invc", [C, 1], BF16, kind="ExternalInput")
    thr = nc.dram_tensor("thr", [1, 1], F32, kind="ExternalInput")
    ident = nc.dram_tensor("ident", [128, 128], BF16, kind="ExternalInput")
    ones2d = nc.dram_tensor("ones2d", [128, 2], BF16, kind="ExternalInput")
    onesrow = nc.dram_tensor("onesrow", [1, 128], F32, kind="ExternalInput")
    zbd = nc.dram_tensor("zbd", [C, 260], BF16, kind="ExternalInput")
    out = nc.dram_tensor("out", [C, L], F32, kind="ExternalOutput")

    x1r = x1[:, :].rearrange("(t p) n -> p t n", p=128)
    x2r = x2[:, :].rearrange("(t p) n -> p t n", p=128)
    outr = out[:, :].rearrange("(t p) n -> p t n", p=128)

    def bcast(ap, p):
        # partition-broadcast of a [1, F] DRAM row to [p, F]
        return bass.AP(tensor=ap.tensor, offset=ap.offset,
                       ap=[[0, p]] + [list(d) for d in ap.ap[1:]])

    with tile.TileContext(nc) as tc:
        with tc.tile_pool(name="consts", bufs=1) as consts, \
             tc.tile_pool(name="resident", bufs=1) as res:
            # ---- constants ----
            wkv_sb = consts.tile([128, 2, 2 * C], BF16)
            for t in range(2):
                nc.sync.dma_start(out=wkv_sb[:, t, :], in_=wkv[:, :].rearrange(
                    "(t p) o -> p t o", p=128)[:, t, :])
            wq_sb = consts.tile([128, 2, C], BF16)
            nc.sync.dma_start(out=wq_sb, in_=wq[:, :].rearrange(
                "(t p) o -> p t o", p=128))
            wfin_sb = consts.tile([128, 2, C], BF16)
            nc.sync.dma_start(out=wfin_sb, in_=wfin[:, :].rearrange(
                "(t p) o -> p t o", p=128))
            w1a_sb = consts.tile([128, 2, 2 * C], BF16)
            w1b_sb = consts.tile([128, 2, 2 * C], BF16)
            w2_sb = consts.tile([128, 4, C], BF16)
            for t in range(2):
                nc.sync.dma_start(out=w1a_sb[:, t, :], in_=w1a[:, :].rearrange(
                    "(t p) o -> p t o", p=128)[:, t, :])
                nc.sync.dma_start(out=w1b_sb[:, t, :], in_=w1b[:, :].rearrange(
                    "(t p) o -> p t o", p=128)[:, t, :])
            for t in range(4):
                nc.sync.dma_start(out=w2_sb[:, t, :], in_=w2[:, :].rearrange(
                    "(t p) o -> p t o", p=128)[:, t, :])
            ident_sb = consts.tile([128, 128], BF16)
            nc.sync.dma_start(out=ident_sb, in_=ident[:, :])

            bq_sb = consts.tile([128, 2], F32)
            bq1_sb = consts.tile([128, 2], F32)
            ba_sb = consts.tile([128, 2], F32)
            for t in range(2):
                sl = slice(t * 128, (t + 1) * 128)
                nc.sync.dma_start(out=bq_sb[:, t:t + 1], in_=bq[sl, :])
                nc.sync.dma_start(out=bq1_sb[:, t:t + 1], in_=bq1[sl, :])
                nc.sync.dma_start(out=ba_sb[:, t:t + 1], in_=ba[sl, :])
            hb_sb = consts.tile([128, 4], F32)
            for t in range(4):
                nc.sync.dma_start(out=hb_sb[:, t:t + 1],
                                  in_=hbv[t * 128:(t + 1) * 128, :])
            invc_sb = consts.tile([128, 2], BF16)
            for t in range(2):
                nc.sync.dma_start(out=invc_sb[:, t:t + 1],
                                  in_=invc[t * 128:(t + 1) * 128, :])
            biask1_bc = consts.tile([128, C], BF16)
            nc.gpsimd.dma_start(out=biask1_bc, in_=bcast(biask1[:, :], 128))
            bvbc = consts.tile([128, C], BF16)
            nc.gpsimd.dma_start(out=bvbc, in_=bcast(bvrow[:, :], 128))
            thr_bc = consts.tile([128, 1], F32)
            nc.gpsimd.dma_start(out=thr_bc, in_=bcast(thr[:, :], 128))
            onesrow_sb = consts.tile([1, 128], F32R)
            nc.sync.dma_start(out=onesrow_sb,
                              in_=onesrow[:, :].bitcast(F32R))
            ones2_sb = consts.tile([128, 2], BF16)
            nc.sync.dma_start(out=ones2_sb, in_=ones2d[:, :])
            lneps = consts.tile([128, 1], F32)
            nc.vector.memset(lneps, LN_EPS)
            eps11 = consts.tile([1, 1], F32)
            nc.vector.memset(eps11, LN_EPS)
            neg1 = consts.tile([128, 1], F32)
            nc.vector.memset(neg1, -1.0)

            # ---- resident activations ----
            x1_sb = res.tile([128, 2, L], BF16)
            q_sb = res.tile([128, 2, L], BF16)
            msgn_sb = res.tile([128, 2, L], BF16)
            kvbd = res.tile([128, 2, 260], BF16)

            # ========= phase 1: x2 side (half N per core + AllReduce) =========
            NT2 = L // 128
            with tc.tile_pool(name="x2p", bufs=3) as x2p, \
                 tc.tile_pool(name="kvbp", bufs=1) as kvbp, \
                 tc.tile_pool(name="sc1", bufs=6) as sc1, \
                 tc.tile_pool(name="dramp", bufs=1, space="DRAM") as dramp, \
                 tc.tile_pool(name="cps", bufs=4, space="PSUM") as cps, \
                 tc.tile_pool(name="kvps", bufs=1, space="PSUM") as kvps:
                kv_ps = [kvps.tile([128, 258], F32, name=f"kv_ps{m}", tag=f"kv{m}")
                         for m in range(2)]
                for ch in range(L // 512):
                    x2t = x2p.tile([128, 2, 512], BF16)
                    for t in range(2):
                        nc.sync.dma_start(
                            out=x2t[:, t, :],
                            in_=x2r[:, t, ch * 512:(ch + 1) * 512])
                    for s in range(4):
                        i = ch * 4 + s
                        cp = cps.tile([128, 2 * C], F32)
                        nc.tensor.matmul(cp, x2t[:, 0, s * 128:(s + 1) * 128],
                                         wkv_sb[:, 0, :], start=True, stop=False)
                        nc.tensor.matmul(cp, x2t[:, 1, s * 128:(s + 1) * 128],
                                         wkv_sb[:, 1, :], start=False, stop=True)
                        kvb = kvbp.tile([128, 514], BF16, name=f"kvbu{i}",
                                        tag=f"kvb{i % 8}")
                        nc.gpsimd.tensor_copy(out=kvb[:, 512:514],
                                              in_=ones2_sb)
                        # K = max(kb1, min(exp(kb1-1), 1)) == elu(cp+bk)+1
                        kb1 = sc1.tile([128, C], BF16)
                        nc.vector.tensor_tensor(
                            out=kb1, in0=cp[:, 0:256], in1=biask1_bc, op=OP.add)
                        ex = sc1.tile([128, C], BF16)
                        nc.scalar.activation(out=ex, in_=kb1, func=AF.Exp,
                                             bias=neg1)
                        nc.vector.scalar_tensor_tensor(
                            out=kvb[:, 0:256], in0=ex, scalar=1.0, in1=kb1,
                            op0=OP.min, op1=OP.max)
                        # V half: plain copy (bias folded at extraction)
                        nc.scalar.copy(out=kvb[:, 256:512],
                                       in_=cp[:, 256:512])
                        nc.tensor.matmul(kv_ps[0], kvb[:, 0:128],
                                         kvb[:, 256:514],
                                         start=(i == 0), stop=(i == NT2 - 1))
                        nc.tensor.matmul(kv_ps[1], kvb[:, 128:256],
                                         kvb[:, 256:514],
                                         start=(i == 0), stop=(i == NT2 - 1))

                # ---- pair AllReduce of partial KV ----
                kvpart = sc1.tile([128, 2, 258], BF16, name="kvpart",
                                  tag="kvpart")
                for t in range(2):
                    nc.vector.tensor_copy(out=kvpart[:, t, :], in_=kv_ps[t])
                cc_in = dramp.tile([C, 258], BF16, name="cc_in")
                cc_out = dramp.tile([C, 258], BF16, name="cc_out")
                nc.sync.dma_start(
                    out=cc_in[:, :].rearrange("(t p) o -> p t o", p=128),
                    in_=kvpart)
                nc.gpsimd.collective_compute(
                    "AllReduce", OP.add,
                    replica_groups=[[0, 1], [2, 3], [4, 5], [6, 7]],
                    ins=[cc_in.opt()], outs=[cc_out.opt()])
                kvsum = sc1.tile([128, 2, 258], BF16, name="kvsum",
                                 tag="kvsum")
                nc.sync.dma_start(
                    out=kvsum,
                    in_=cc_out[:, :].rearrange("(t p) o -> p t o", p=128))

                # ---- KVbd / KsumBD extraction (V bias folded as Ksum x bv) ----
                nc.sync.dma_start(out=kvbd, in_=zbd[:, :].rearrange(
                    "(t p) o -> p t o", p=128))
                ksum_sb = sc1.tile([128, 2], F32)
                for t in range(2):
                    nc.vector.tensor_copy(out=ksum_sb[:, t:t + 1],
                                          in_=kvsum[:, t, 256:257])
                for t in range(2):
                    for hh in range(2):
                        h = t * 2 + hh
                        rsl = slice(hh * 64, hh * 64 + 64)
                        csl = slice(h * 64, h * 64 + 64)
                        nc.vector.scalar_tensor_tensor(
                            out=kvbd[rsl, t, csl], in0=bvbc[rsl, csl],
                            scalar=ksum_sb[rsl, t:t + 1],
                            in1=kvsum[rsl, t, csl],
                            op0=OP.mult, op1=OP.add)
                        nc.vector.tensor_copy(
                            out=kvbd[rsl, t, 256 + h:257 + h],
                            in_=kvsum[rsl, t, 256:257])

            # ============ phase 2+3: q conv, msg, LN1 ============
            with tc.tile_pool(name="sc2", bufs=4) as sc2, \
                 tc.tile_pool(name="sc3", bufs=6) as sc3, \
                 tc.tile_pool(name="stat", bufs=4) as stat, \
                 tc.tile_pool(name="qaps", bufs=2, space="PSUM") as qaps, \
                 tc.tile_pool(name="msgps", bufs=1, space="PSUM") as msgps, \
                 tc.tile_pool(name="trps", bufs=2, space="PSUM") as trps:
                for ch in range(NCH):
                    sl = slice(ch * 512, (ch + 1) * 512)
                    for t in range(2):
                        nc.sync.dma_start(out=x1_sb[:, t, sl],
                                          in_=x1r[:, t, sl])
                    for m in range(2):
                        mc = slice(m * 128, (m + 1) * 128)
                        qp = qaps.tile([128, 512], F32)
                        nc.tensor.matmul(qp, wq_sb[:, 0, mc],
                                         x1_sb[:, 0, sl], start=True, stop=False)
                        nc.tensor.matmul(qp, wq_sb[:, 1, mc],
                                         x1_sb[:, 1, sl], start=False, stop=True)
                        ex2 = sc2.tile([128, 512], BF16)
                        nc.scalar.activation(out=ex2, in_=qp, func=AF.Exp,
                                             bias=bq_sb[:, m:m + 1])
                        mn2 = sc2.tile([128, 512], BF16)
                        nc.vector.tensor_scalar(
                            out=mn2, in0=ex2, scalar1=1.0, scalar2=None,
                            op0=OP.min)
                        nc.vector.scalar_tensor_tensor(
                            out=q_sb[:, m, sl], in0=qp,
                            scalar=bq1_sb[:, m:m + 1], in1=mn2,
                            op0=OP.add, op1=OP.max)
                    zsc = stat.tile([128, 16], F32, name="zsc", tag="zsc")
                    mps = []
                    for s_ in range(4):
                        l0 = ch * 512 + s_ * 128
                        lsl = slice(l0, l0 + 128)
                        mp = msgps.tile([128, 260], F32, name=f"mp{s_}",
                                        tag=f"mp{s_}")
                        nc.tensor.matmul(mp, q_sb[:, 0, lsl], kvbd[:, 0, :],
                                         start=True, stop=False)
                        nc.tensor.matmul(mp, q_sb[:, 1, lsl], kvbd[:, 1, :],
                                         start=False, stop=True)
                        nc.vector.tensor_copy(out=zsc[:, s_ * 4:s_ * 4 + 4],
                                              in_=mp[:, 256:260])
                        mps.append(mp)
                    mk = stat.tile([128, 16], F32, name="mk", tag="mk")
                    nc.vector.tensor_scalar(
                        out=mk, in0=zsc, scalar1=thr_bc,
                        scalar2=None, op0=OP.is_gt)
                    nc.vector.tensor_tensor(out=mk, in0=zsc, in1=mk,
                                            op=OP.mult)
                    nc.vector.tensor_scalar(
                        out=mk, in0=mk, scalar1=ATTN_EPS, scalar2=None,
                        op0=OP.add)
                    zt = stat.tile([128, 16], F32, name="zt", tag="zt")
                    nc.vector.reciprocal(out=zt, in_=mk)
                    sums = stat.tile([128, 4], F32, name="sums", tag="sums")
                    sumsq = stat.tile([128, 4], F32, name="sumsq", tag="sumsq")
                    mss = []
                    for s_ in range(4):
                        mp = mps[s_]
                        ms = sc3.tile([128, C], BF16, name="ms", tag=f"ms{s_}")
                        zb = zt[:, s_ * 4:s_ * 4 + 4]
                        zb = bass.AP(tensor=zb.tensor, offset=zb.offset,
                                     ap=[list(zb.ap[0]), list(zb.ap[1]),
                                         [0, 64]])
                        nc.vector.scalar_tensor_tensor(
                            out=ms.rearrange("p (h d) -> p h d", h=4),
                            in0=mp[:, 0:256].rearrange("p (h d) -> p h d", h=4),
                            scalar=0.0, in1=zb, op0=OP.add, op1=OP.mult,
                            accum_out=sums[:, s_:s_ + 1])
                        sqj = sc3.tile([128, C], BF16, name="sqj", tag="sqj")
                        nc.scalar.activation(out=sqj, in_=ms, func=AF.Square,
                                             accum_out=sumsq[:, s_:s_ + 1])
                        mss.append(ms)
                    mu = stat.tile([128, 4], F32, name="mu", tag="mu")
                    nc.vector.tensor_scalar(out=mu, in0=sums, scalar1=1.0 / C,
                                            scalar2=None, op0=OP.mult)
                    musq = stat.tile([128, 4], F32, name="musq", tag="musq")
                    nc.vector.tensor_tensor(out=musq, in0=mu, in1=mu,
                                            op=OP.mult)
                    varq = stat.tile([128, 4], F32, name="varq", tag="varq")
                    nc.vector.scalar_tensor_tensor(
                        out=varq, in0=sumsq, scalar=1.0 / C, in1=musq,
                        op0=OP.mult, op1=OP.subtract)
                    sdv = stat.tile([128, 4], F32, name="sdv", tag="sdv")
                    nc.scalar.activation(out=sdv, in_=varq, func=AF.Sqrt,
                                         bias=lneps)
                    rz = stat.tile([128, 4], F32, name="rz", tag="rz")
                    nc.vector.reciprocal(out=rz, in_=sdv)
                    for s_ in range(4):
                        l0 = ch * 512 + s_ * 128
                        lsl = slice(l0, l0 + 128)
                        msn = sc3.tile([128, C], BF16, name="msn", tag="msn")
                        nc.vector.tensor_scalar(
                            out=msn, in0=mss[s_], scalar1=mu[:, s_:s_ + 1],
                            scalar2=rz[:, s_:s_ + 1],
                            op0=OP.subtract, op1=OP.mult)
                        for t in range(2):
                            tp = trps.tile([128, 128], BF16)
                            nc.tensor.transpose(
                                tp, msn[:, t * 128:(t + 1) * 128], ident_sb)
                            if t == 0:
                                nc.scalar.copy(out=msgn_sb[:, t, lsl], in_=tp)
                            else:
                                nc.vector.tensor_copy(out=msgn_sb[:, t, lsl],
                                                      in_=tp)

            # ================= phase 4: MLP + LN2 + final =================
            with tc.tile_pool(name="hpool", bufs=5) as hpool, \
                 tc.tile_pool(name="sc4", bufs=4) as sc4, \
                 tc.tile_pool(name="st4", bufs=2) as st4, \
                 tc.tile_pool(name="outp", bufs=3) as outp, \
                 tc.tile_pool(name="hps", bufs=2, space="PSUM") as hps, \
                 tc.tile_pool(name="o2ps", bufs=1, space="PSUM") as o2ps, \
                 tc.tile_pool(name="stps", bufs=1, space="PSUM") as stps, \
                 tc.tile_pool(name="rbps", bufs=1, space="PSUM") as rbps, \
                 tc.tile_pool(name="augps", bufs=2, space="PSUM") as augps:
                for ch in range(NCH):
                    sl = slice(ch * 512, (ch + 1) * 512)
                    hsb = []
                    for m in range(4):
                        mc = slice(m * 128, (m + 1) * 128)
                        hp = hps.tile([128, 512], F32)
                        nc.tensor.matmul(hp, w1a_sb[:, 0, mc], x1_sb[:, 0, sl],
                                         start=True, stop=False)
                        nc.tensor.matmul(hp, w1a_sb[:, 1, mc], x1_sb[:, 1, sl],
                                         start=False, stop=False)
                        nc.tensor.matmul(hp, w1b_sb[:, 0, mc],
                                         msgn_sb[:, 0, sl],
                                         start=False, stop=False)
                        nc.tensor.matmul(hp, w1b_sb[:, 1, mc],
                                         msgn_sb[:, 1, sl],
                                         start=False, stop=True)
                        ht = hpool.tile([128, 512], BF16)
                        nc.scalar.activation(out=ht, in_=hp, func=AF.Relu,
                                             bias=hb_sb[:, m:m + 1])
                        hsb.append(ht)
                    o2p = [o2ps.tile([128, 512], F32, name=f"o2p{m2}",
                                     tag=f"o2_{m2}")
                           for m2 in range(2)]
                    for m2 in range(2):
                        mc2 = slice(m2 * 128, (m2 + 1) * 128)
                        for k in range(4):
                            nc.tensor.matmul(o2p[m2], w2_sb[:, k, mc2], hsb[k],
                                             start=(k == 0), stop=(k == 3))
                    # LN2: mean already folded into W2; var = sum(invc * z^2)
                    ssq = stps.tile([1, 512], F32, tag="ssq")
                    for m2 in range(2):
                        sqs = sc4.tile([128, 512], BF16, name=f"sqs{m2}",
                                       tag=f"sqs{m2}")
                        nc.scalar.activation(out=sqs, in_=o2p[m2],
                                             func=AF.Square)
                        nc.tensor.matmul(ssq, invc_sb[:, m2:m2 + 1], sqs,
                                         start=(m2 == 0), stop=(m2 == 1))
                    sd_row = st4.tile([1, 512], F32, name="sd_row", tag="sd")
                    nc.scalar.activation(out=sd_row, in_=ssq, func=AF.Sqrt,
                                         bias=eps11)
                    rstd_row = st4.tile([1, 512], F32, name="rstd_row",
                                        tag="rstd")
                    nc.vector.reciprocal_approx_fast(out=rstd_row, in_=sd_row)
                    rstd_r = st4.tile([1, 512], F32R, name="rstd_r",
                                      tag="rstd_r")
                    nc.vector.tensor_copy(out=rstd_r, in_=rstd_row)
                    rbc = rbps.tile([128, 512], F32, tag="rbc")
                    nc.tensor.matmul(rbc, onesrow_sb, rstd_r,
                                     start=True, stop=True)
                    rbc_sb = sc4.tile([128, 512], BF16, name="rbc_sb",
                                      tag="rbc_sb")
                    nc.vector.tensor_copy(out=rbc_sb, in_=rbc)
                    for m2 in range(2):
                        ap_ = augps.tile([128, 512], F32)
                        nc.tensor.matmul(ap_, wfin_sb[:, 0, m2 * 128:(m2 + 1) * 128],
                                         x1_sb[:, 0, sl], start=True, stop=False)
                        nc.tensor.matmul(ap_, wfin_sb[:, 1, m2 * 128:(m2 + 1) * 128],
                                         x1_sb[:, 1, sl], start=False, stop=True)
                        t1 = sc4.tile([128, 512], BF16, name=f"t1_{m2}",
                                      tag=f"t1_{m2}")
                        nc.vector.tensor_tensor(out=t1, in0=o2p[m2],
                                                in1=rbc_sb, op=OP.mult)
                        ot = outp.tile([128, 512], F32)
                        nc.vector.scalar_tensor_tensor(
                            out=ot, in0=ap_, scalar=ba_sb[:, m2:m2 + 1], in1=t1,
                            op0=OP.add, op1=OP.add)
                        nc.sync.dma_start(out=outr[:, m2, sl], in_=ot)

    nc.compile()
    return nc


def _host_prep(inputs):
    """Fold BN/LN affine params into weights; build per-core input maps."""
    f32 = np.float32
    x1 = np.asarray(inputs["x1"], f32)
    x2 = np.asarray(inputs["x2"], f32)
    Wq, bq = np.asarray(inputs["Wq"], f32), np.asarray(inputs["bq"], f32)
    Wk, bk = np.asarray(inputs["Wk"], f32), np.asarray(inputs["bk"], f32)
    Wv, bv = np.asarray(inputs["Wv"], f32), np.asarray(inputs["bv"], f32)
    W1, W2 = np.asarray(inputs["W1"], f32), np.asarray(inputs["W2"], f32)
    g1, b1 = np.asarray(inputs["g1"], f32), np.asarray(inputs["b1"], f32)
    g2, b2 = np.asarray(inputs["g2"], f32), np.asarray(inputs["b2"], f32)
    Wa, ba = np.asarray(inputs["Wa"], f32), np.asarray(inputs["ba"], f32)
    bn_g, bn_b = np.asarray(inputs["bn_g"], f32), np.asarray(inputs["bn_b"], f32)
    bn_m, bn_v = np.asarray(inputs["bn_m"], f32), np.asarray(inputs["bn_v"], f32)
    thr = np.asarray(inputs["threshold"], f32)

    c = lambda a: np.ascontiguousarray(a, dtype=f32)
    cb = lambda a: np.ascontiguousarray(np.asarray(a, f32), dtype=BFNP)
    c8 = lambda a: np.ascontiguousarray(np.asarray(a, f32), dtype=F8NP)

    wkv = np.concatenate([Wk.T, Wv.T], axis=1)               # [C, 2C]
    scale_bn = bn_g / np.sqrt(bn_v + BN_EPS)
    wfin = (scale_bn[:, None] * Wa).T + np.eye(C, dtype=f32)  # aug + x1 fold
    ba_f = scale_bn * ba + (bn_b - bn_m * scale_bn) + b2
    W1a, W1b = W1[:, :C], W1[:, C:]
    w1a = W1a.T                                              # [C, 2C]
    w1b = (W1b * g1[None, :]).T                              # [C, 2C]
    hb = (W1b @ b1)[:, None]                                 # [2C, 1]
    # W2 with LN2 mean-centering and g2 folded in (per output channel)
    w2t = W2.T                                               # [2C, C]
    w2pp = (w2t - w2t.mean(axis=1, keepdims=True)) * g2[None, :]
    invc = (1.0 / (C * g2 * g2))[:, None]                    # [C, 1]
    shared = {
        "wkv": cb(wkv),
        "wq": cb(Wq.T), "bq": c(bq[:, None]), "bq1": c(bq[:, None] + 1.0),
        "wfin": cb(wfin), "ba": c(ba_f[:, None]),
        "w1a": cb(w1a), "w1b": cb(w1b), "hb": c(hb),
        "w2": cb(w2pp),
        "biask1": cb((bk + 1.0)[None, :]), "bvrow": cb(bv[None, :]),
        "invc": cb(invc),
        "thr": c(thr.reshape(1, 1)),
        "ident": cb(np.eye(128, dtype=f32)),
        "ones2d": cb(np.ones((128, 2), dtype=f32)),
        "onesrow": c(np.ones((1, 128), dtype=f32)),
        "zbd": cb(np.zeros((C, 260), dtype=f32)),
    }
    x1b = np.ascontiguousarray(x1, dtype=BFNP)
    x2b = np.ascontiguousarray(x2, dtype=BFNP)
    in_maps = []
    for core in range(8):
        b_, half = core // 2, core % 2
        m = dict(shared)
        m["x1"] = np.ascontiguousarray(x1b[b_][:, half * L:(half + 1) * L])
        m["x2"] = np.ascontiguousarray(
            x2b[b_][:, half * L:(half + 1) * L])
        in_maps.append(m)
    return in_maps


def _get_nc():
    if "nc" not in _CACHE:
        _CACHE["nc"] = _build()
    return _CACHE["nc"]


def kernel(**inputs) -> np.ndarray:
    nc = _get_nc()
    in_maps = _host_prep(inputs)
    res = run_bass_kernel_spmd(nc, in_maps, core_ids=list(range(8)),
                               trace=bool(int(os.environ.get("KBENCH_TRACE", "0"))))
    if os.environ.get("KBENCH_TIME_OUT"):
        with open(os.environ["KBENCH_TIME_OUT"], "w") as f:
            f.write(str(res.exec_time_ns))
    out = np.empty((B, C, N), np.float32)
    for core in range(8):
        b_, half = core // 2, core % 2
        out[b_][:, half * L:(half + 1) * L] = res.results[core]["out"]
    return out
